# revision 1
# baseline (speedup 1.0000x reference)
"""Trainium2 Bass kernel for nn_SpaceTimeAtten (space-time attention block).

Contract: kernel(**inputs) takes FULL unsharded numpy inputs (see reference
setup_inputs) and returns the FULL (2, 512, 8, 28, 28) float32 output.

Sharding: 8 cores = 2 batches x 4 query-chunks. Each core:
  - computes Q projection (ph_x) for its local t-range,
  - computes K/V projections (pg, ph_m^T) for the full (padded) s-range,
  - runs attention with the energy matrix built TRANSPOSED
    (E^T = [s_partitions, t_free]) so that exp(E^T - M1) is directly the
    lhsT operand of the PV matmul -- no on-device transposes of the big
    attention matrix. M1 is a host-estimated global upper bound of the
    energy max (any constant shift cancels exactly in softmax).
  - row-sums r_t of exp come from free-dim-1 matmuls against a ones vector.
  - the second softmax (over t, per channel) and BatchNorm need global
    reductions: one 8-core AllReduce of a [128,16] stats tile carries both
    batches' softmax denominators and the BN sum/sumsq.
"""

import numpy as np

# ---- problem constants (hardcoded per contract) ----
N_B, C, T, H, W = 2, 512, 8, 28, 28
THW = T * H * W            # 6272
BN_EPS = 1e-5

CI = 4                     # input-channel 128-chunks
CO = 4                     # output-channel 128-chunks
S_PAD = 6272               # 49 s-tiles of 128 (exact, no padding)
NST = 49
S_TILES_H = (25, 24)       # s-tiles per resident half
S_BASE_H = (0, 3200)
T_LOC = 1664               # local t per core (13 tiles of 128)
NTT = 13
BLOCKS = [(0, 4), (4, 4), (8, 3), (11, 2)]   # (t-tile start, n tiles)
R_EPS = 1e-30

_PROG_CACHE = {}


def _build_program(m1, m2, gamma, debug=False):
    import concourse.bass as bass
    import concourse.mybir as mybir
    import concourse.tile as tile
    from concourse import bacc

    # constants duplicated here so this module stays import-light
    N_B, C = 2, 512
    THW = 6272
    BN_EPS = 1e-5
    CI = CO = 4
    S_PAD = 6272
    S_TILES_H = (25, 24)
    S_BASE_H = (0, 3200)
    T_LOC = 1664
    NTT = 13
    BLOCKS = [(0, 4), (4, 4), (8, 3), (11, 2)]
    R_EPS = 1e-30

    f32 = mybir.dt.float32
    f32r = mybir.dt.float32r
    bf16 = mybir.dt.bfloat16
    EXP = mybir.ActivationFunctionType.Exp
    SQRT = mybir.ActivationFunctionType.Sqrt
    AX = mybir.AxisListType.X
    MUL = mybir.AluOpType.mult
    ADD = mybir.AluOpType.add

    nc = bacc.Bacc("TRN2")

    x_full = nc.dram_tensor("x_full", [C, S_PAD], f32r, kind="ExternalInput")
    mask_full = nc.dram_tensor("mask_full", [C, S_PAD], f32r, kind="ExternalInput")
    x_loc = nc.dram_tensor("x_loc", [C, T_LOC], f32, kind="ExternalInput")
    wht = nc.dram_tensor("wht", [C, C], f32r, kind="ExternalInput")
    wgt = nc.dram_tensor("wgt", [C, C], f32r, kind="ExternalInput")
    wmt = nc.dram_tensor("wmt", [C, C], f32r, kind="ExternalInput")
    wzt = nc.dram_tensor("wzt", [C, C], f32, kind="ExternalInput")
    bh_in = nc.dram_tensor("bh_in", [128, CO], f32, kind="ExternalInput")
    bg_in = nc.dram_tensor("bg_in", [128, CO], f32, kind="ExternalInput")
    bm_in = nc.dram_tensor("bm_in", [128, CO], f32, kind="ExternalInput")
    bz_in = nc.dram_tensor("bz_in", [128, CO], f32, kind="ExternalInput")
    bh_row_in = nc.dram_tensor("bh_row_in", [128, C], f32, kind="ExternalInput")
    bnw_in = nc.dram_tensor("bnw_in", [128, CO], f32, kind="ExternalInput")
    bnb_in = nc.dram_tensor("bnb_in", [128, CO], f32, kind="ExternalInput")
    ones_in = nc.dram_tensor("ones_in", [128, 1], bf16, kind="ExternalInput")
    tmaddp_in = nc.dram_tensor("tmaddp_in", [128, 16], f32, kind="ExternalInput")
    bzc_in = nc.dram_tensor("bzc_in", [128, 8], f32, kind="ExternalInput")
    bsel_in = nc.dram_tensor("bsel_in", [128, 2], f32, kind="ExternalInput")

    out_loc = nc.dram_tensor("out_loc", [C, T_LOC], f32, kind="ExternalOutput")
    if debug:
        d_phx = nc.dram_tensor("d_phx", [C, T_LOC], f32, kind="ExternalOutput")
        d_z = nc.dram_tensor("d_z", [C, T_LOC], f32, kind="ExternalOutput")
        d_r = nc.dram_tensor("d_r", [128, 16], f32, kind="ExternalOutput")
        d_wy = nc.dram_tensor("d_wy", [C, T_LOC], f32, kind="ExternalOutput")

    cc_in = nc.dram_tensor("cc_in", [128, 16], f32)
    cc_out = nc.dram_tensor("cc_out", [128, 16], f32)

    def dview(dram):
        return dram.rearrange("(k p) s -> p k s", p=128)

    with tile.TileContext(nc) as tc:
        with (
            tc.tile_pool(name="const", bufs=1) as cpool,
            tc.tile_pool(name="ptile", bufs=4) as ptpool,
            tc.tile_pool(name="metile", bufs=2) as mepool,
            tc.tile_pool(name="small", bufs=1) as spool,
        ):
            # ---- constants ----
            ones_t = cpool.tile([128, 1], bf16, tag="ones")
            nc.gpsimd.dma_start(out=ones_t[:], in_=ones_in[:])
            bh_t = cpool.tile([128, CO], f32, tag="bh")
            bg_t = cpool.tile([128, CO], f32, tag="bg")
            bm_t = cpool.tile([128, CO], f32, tag="bm")
            bz_t = cpool.tile([128, CO], f32, tag="bz")
            bnw_t = cpool.tile([128, CO], f32, tag="bnw")
            bnb_t = cpool.tile([128, CO], f32, tag="bnb")
            for tl, dr in ((bh_t, bh_in), (bg_t, bg_in), (bm_t, bm_in),
                           (bz_t, bz_in), (bnw_t, bnw_in), (bnb_t, bnb_in)):
                nc.gpsimd.dma_start(out=tl[:], in_=dr[:])
            bh_row = cpool.tile([128, C], f32, tag="bhrow")
            nc.gpsimd.dma_start(out=bh_row[:], in_=bh_row_in[:])
            bsel_t = cpool.tile([128, 2], f32, tag="bsel")
            nc.gpsimd.dma_start(out=bsel_t[:], in_=bsel_in[:])
            tmaddp = cpool.tile([128, 16], f32, tag="tmaddp")
            nc.gpsimd.dma_start(out=tmaddp[:], in_=tmaddp_in[:])
            bzc_t = cpool.tile([128, 8], f32, tag="bzc")
            nc.gpsimd.dma_start(out=bzc_t[:], in_=bzc_in[:])
            m1b = cpool.tile([128, 1], f32, tag="m1b")
            nc.vector.memset(m1b[:], -m1)
            m2b = cpool.tile([128, 1], f32, tag="m2b")
            nc.vector.memset(m2b[:], -m2)
            one_f = cpool.tile([1, 1], f32, tag="onef")
            nc.vector.memset(one_f[:], 1.0)

            FC = T_LOC // 4  # 416

            # ---- weights (gpsimd queue; piece DMAs go on sync queue) ----
            p_w1 = tc.alloc_tile_pool(name="w1", bufs=1)
            wt_h = p_w1.tile([128, CI, C], f32r, tag="wh")
            wt_g = p_w1.tile([128, CI, C], f32r, tag="wg")
            for ci in range(CI):
                eng = nc.gpsimd if ci % 2 == 0 else nc.sync
                eng.dma_start(out=wt_g[:, ci, :], in_=dview(wgt)[:, ci, :])
            for ci in range(CI):
                eng = nc.gpsimd if ci % 2 == 1 else nc.sync
                eng.dma_start(out=wt_h[:, ci, :], in_=dview(wht)[:, ci, :])

            p_phx = tc.alloc_tile_pool(name="phxp", bufs=1)
            phx = p_phx.tile([128, CI, T_LOC], f32r, tag="phx")

            p_acc = tc.alloc_tile_pool(name="accp", bufs=1, side="right")
            acc = p_acc.tile([128, NTT, 512], f32, tag="acc")
            racc_row = p_acc.tile([1, T_LOC], f32, tag="racc")

            p_kv = tc.alloc_tile_pool(name="kvp", bufs=1)
            p_piece = tc.alloc_tile_pool(name="piecep", bufs=2)

            for h in range(2):
                s_base = S_BASE_H[h]
                n_st = S_TILES_H[h]
                s_cols = n_st * 128
                pgh = p_kv.tile([128, CI, S_TILES_H[0] * 128], f32r, tag="pgh",
                                name=f"pgh{h}")
                phmh = p_kv.tile([128, S_TILES_H[0], C], bf16, tag="phmh",
                                 name=f"phmh{h}")

                # -- K/V conv phase (scoped PSUM pool); pieces of up to 4 s-tiles --
                ps_c = tc.alloc_tile_pool(name=f"psc{h}", bufs=2, space="PSUM")
                pieces = []
                o = 0
                while o < n_st:
                    w = min(4, n_st - o)
                    pieces.append((o, w))
                    o += w
                for (pt0, ptw) in pieces:
                    s_off = pt0 * 128
                    pw = ptw * 128
                    xp = p_piece.tile([128, CI, 512], f32r, tag="piece",
                                      name="xp")
                    nc.sync.dma_start(
                        out=xp[:, :, :pw],
                        in_=dview(x_full)[:, :, s_base + s_off:s_base + s_off + pw])
                    for co in range(CO):
                        ps = ps_c.tile([128, 512], f32, tag="c")
                        for ci in range(CI):
                            nc.tensor.matmul(
                                ps[:, :pw],
                                wt_g[:, ci, co * 128:(co + 1) * 128],
                                xp[:, ci, :pw],
                                start=(ci == 0), stop=(ci == CI - 1))
                        nc.vector.tensor_scalar_add(
                            pgh[:, co, s_off:s_off + pw],
                            ps[:, :pw], bg_t[:, co:co + 1])
                    mp = p_piece.tile([128, CI, 512], f32r, tag="piece",
                                      name="mp")
                    nc.gpsimd.dma_start(
                        out=mp[:, :, :pw],
                        in_=dview(mask_full)[:, :, s_base + s_off:s_base + s_off + pw])
                    for sj in range(ptw):
                        st = pt0 + sj
                        ps = ps_c.tile([128, 512], f32, tag="c")
                        for ci in range(CI):
                            nc.tensor.matmul(
                                ps[:],
                                mp[:, ci, sj * 128:(sj + 1) * 128],
                                wt_h[:, ci, :],
                                start=(ci == 0), stop=(ci == CI - 1))
                        nc.vector.tensor_add(phmh[:, st, :], ps[:], bh_row[:])

                if h == 0:
                    # Q projection, after the piece convs so small DMAs win the
                    # queue at kernel start
                    p_xl = tc.alloc_tile_pool(name="xlp", bufs=1)
                    xloc_t = p_xl.tile([128, CI, T_LOC], f32r, tag="xloc")
                    nc.sync.dma_start(out=xloc_t[:],
                                      in_=dview(x_loc).bitcast(f32r))
                    for co in range(CO):
                        for fc in range(4):
                            ps = ps_c.tile([128, 512], f32, tag="c")
                            for ci in range(CI):
                                nc.tensor.matmul(
                                    ps[:, :FC],
                                    wt_h[:, ci, co * 128:(co + 1) * 128],
                                    xloc_t[:, ci, fc * FC:(fc + 1) * FC],
                                    start=(ci == 0), stop=(ci == CI - 1))
                            nc.vector.tensor_scalar_add(
                                phx[:, co, fc * FC:(fc + 1) * FC],
                                ps[:, :FC], bh_t[:, co:co + 1])
                    p_xl.release()
                    if debug:
                        nc.sync.dma_start(out=dview(d_phx).bitcast(f32r),
                                          in_=phx[:])
                ps_c.release()

                # -- attention (scoped PSUM: e:2 + o:4 + r:1 = 7 banks) --
                ps_att = tc.alloc_tile_pool(name=f"psa{h}", bufs=1, space="PSUM")
                for bi, (t0, nt) in enumerate(BLOCKS):
                    tfree = nt * 128
                    ops = [ps_att.tile([128, 512], f32, tag=f"o{j}", name=f"o{j}")
                           for j in range(nt)]
                    rps = ps_att.tile([1, 512], f32, tag="r", name="rps")
                    for st in range(n_st):
                        eps_t = ps_att.tile([128, 512], f32, tag="e", bufs=2,
                                            name="eps")
                        for ci in range(CI):
                            nc.tensor.matmul(
                                eps_t[:, :tfree],
                                pgh[:, ci, st * 128:(st + 1) * 128],
                                phx[:, ci, t0 * 128:t0 * 128 + tfree],
                                start=(ci == 0), stop=(ci == CI - 1))
                        pt = ptpool.tile([128, 512], bf16, tag="pt")
                        nc.scalar.activation(pt[:, :tfree], eps_t[:, :tfree],
                                             EXP, bias=m1b[:], scale=1.0)
                        for j in range(nt):
                            nc.tensor.matmul(
                                ops[j][:],
                                pt[:, j * 128:(j + 1) * 128],
                                phmh[:, st, :],
                                start=(st == 0), stop=(st == n_st - 1))
                        nc.tensor.matmul(
                            rps[:, :tfree],
                            ones_t[:],
                            pt[:, :tfree],
                            start=(st == 0), stop=(st == n_st - 1))
                    for j in range(nt):
                        tt = t0 + j
                        if h == 0:
                            nc.vector.tensor_copy(acc[:, tt, :], ops[j][:])
                        else:
                            nc.vector.tensor_add(acc[:, tt, :], acc[:, tt, :],
                                                 ops[j][:])
                    rsl = racc_row[0:1, t0 * 128:t0 * 128 + tfree]
                    if h == 0:
                        nc.vector.tensor_copy(rsl, rps[0:1, :tfree])
                    else:
                        nc.vector.tensor_add(rsl, rsl, rps[0:1, :tfree])
                ps_att.release()

            p_piece.release()
            p_kv.release()
            p_phx.release()
            p_w1.release()

            # ======== P3: r gather + normalize + transpose to [c, t] ========
            ident = cpool.tile([128, 128], f32, tag="ident")
            from concourse.masks import make_identity
            make_identity(nc, ident[:])
            p_z = tc.alloc_tile_pool(name="zp", bufs=1)
            z_t = p_z.tile([128, CO, T_LOC], f32, tag="z")
            ps_t3 = tc.alloc_tile_pool(name="pst3", bufs=2, space="PSUM")
            rrec = spool.tile([128, 16], f32, tag="rrec")
            for tt in range(NTT):
                tpr = ps_t3.tile([128, 512], f32, tag="t3", name="tpr")
                nc.tensor.matmul(tpr[:, 0:1],
                                 racc_row[0:1, tt * 128:(tt + 1) * 128],
                                 one_f[:], start=True, stop=True)
                nc.vector.tensor_copy(rrec[:, tt:tt + 1], tpr[:, 0:1])
            if debug:
                nc.sync.dma_start(out=d_r[:], in_=rrec[:])
            nc.vector.tensor_scalar_add(rrec[:], rrec[:], R_EPS)
            nc.vector.reciprocal(rrec[:], rrec[:])
            for tt in range(NTT):
                me = mepool.tile([128, 512], f32, tag="me")
                nc.vector.tensor_scalar(me[:], acc[:, tt, :],
                                        rrec[:, tt:tt + 1], tmaddp[:, tt:tt + 1],
                                        op0=MUL, op1=ADD)
                for co in range(CO):
                    tp = ps_t3.tile([128, 512], f32, tag="t3", name="tp")
                    nc.tensor.transpose(tp[:, :128], me[:, co * 128:(co + 1) * 128],
                                        ident[:])
                    nc.vector.tensor_copy(z_t[:, co, tt * 128:(tt + 1) * 128],
                                          tp[:, :128])
            ps_t3.release()
            p_acc.release()
            if debug:
                nc.sync.dma_start(out=dview(d_z), in_=z_t[:])

            # ======== P4a: second-softmax exp + local sums ========
            p_expz = tc.alloc_tile_pool(name="expzp", bufs=1)
            expz = p_expz.tile([128, CO, T_LOC], f32, tag="expz")
            se_loc = spool.tile([128, CO], f32, tag="seloc")
            for co in range(CO):
                nc.scalar.activation(expz[:, co, :], z_t[:, co, :], EXP,
                                     bias=m2b[:], scale=1.0,
                                     accum_out=se_loc[:, co:co + 1])
            stats = spool.tile([128, 16], f32, tag="stats")
            nc.vector.tensor_scalar_mul(stats[:, 0:CO], se_loc[:], bsel_t[:, 0:1])
            nc.vector.tensor_scalar_mul(stats[:, CO:2 * CO], se_loc[:],
                                        bsel_t[:, 1:2])

            # ======== P3.5: wy conv + BN partials, collective, pm conv ========
            p_w2 = tc.alloc_tile_pool(name="w2", bufs=1)
            wt_m = p_w2.tile([128, CI, C], f32r, tag="wm")
            wt_z = p_w2.tile([128, CI, C], f32, tag="wz")
            nc.gpsimd.dma_start(out=wt_m[:], in_=dview(wmt))
            nc.gpsimd.dma_start(out=wt_z[:], in_=dview(wzt))
            p_pmwy = tc.alloc_tile_pool(name="pmwyp", bufs=1, side="right")
            pm_t = p_pmwy.tile([128, CO, T_LOC], f32, tag="pm")
            wy_t = p_pmwy.tile([128, CO, T_LOC], f32, tag="wy")
            p_xl2 = tc.alloc_tile_pool(name="xlp2", bufs=1)
            xloc2r = p_xl2.tile([128, CI, T_LOC], f32r, tag="xloc2r")
            nc.sync.dma_start(out=xloc2r[:], in_=dview(x_loc).bitcast(f32r))
            xloc2 = p_xl2.tile([128, CI, T_LOC], f32, tag="xloc2")
            nc.sync.dma_start(out=xloc2[:], in_=dview(x_loc))
            ps_c2 = tc.alloc_tile_pool(name="psc2", bufs=2, space="PSUM")
            p_scr = tc.alloc_tile_pool(name="scrp", bufs=2)
            for co in range(CO):
                for fc in range(4):
                    ps = ps_c2.tile([128, 512], f32, tag="c")
                    for ci in range(CI):
                        nc.tensor.matmul(
                            ps[:, :FC],
                            wt_z[:, ci, co * 128:(co + 1) * 128],
                            xloc2[:, ci, fc * FC:(fc + 1) * FC],
                            start=(ci == 0), stop=(ci == CI - 1))
                    nc.vector.tensor_scalar_add(
                        wy_t[:, co, fc * FC:(fc + 1) * FC],
                        ps[:, :FC], bz_t[:, co:co + 1])
                nc.vector.reduce_sum(stats[:, 8 + co:9 + co], wy_t[:, co, :],
                                     axis=AX)
                scr = p_scr.tile([128, T_LOC], f32, tag="scr")
                nc.vector.tensor_mul(scr[:], wy_t[:, co, :], wy_t[:, co, :])
                nc.vector.reduce_sum(stats[:, 12 + co:13 + co], scr[:], axis=AX)
            nc.sync.dma_start(out=cc_in[:], in_=stats[:])
            nc.gpsimd.collective_compute(
                "AllReduce", mybir.AluOpType.add,
                replica_groups=[[0, 1, 2, 3, 4, 5, 6, 7]],
                ins=[cc_in[:]], outs=[cc_out[:]])
            for co in range(CO):
                for fc in range(4):
                    ps = ps_c2.tile([128, 512], f32, tag="c")
                    for ci in range(CI):
                        nc.tensor.matmul(
                            ps[:, :FC],
                            wt_m[:, ci, co * 128:(co + 1) * 128],
                            xloc2r[:, ci, fc * FC:(fc + 1) * FC],
                            start=(ci == 0), stop=(ci == CI - 1))
                    nc.vector.tensor_scalar_add(
                        pm_t[:, co, fc * FC:(fc + 1) * FC],
                        ps[:, :FC], bm_t[:, co:co + 1])
            ps_c2.release()
            p_scr.release()
            p_xl2.release()
            p_w2.release()
            if debug:
                nc.sync.dma_start(out=dview(d_wy), in_=wy_t[:])

            # mt0 = expz * pm — independent of the collective result
            p_mt0 = tc.alloc_tile_pool(name="mt0p", bufs=1)
            mt0 = p_mt0.tile([128, CO, T_LOC], f32, tag="mt0")
            for co in range(CO):
                nc.vector.tensor_mul(mt0[:, co, :], expz[:, co, :], pm_t[:, co, :])

            gst = spool.tile([128, 16], f32, tag="gst")
            nc.sync.dma_start(out=gst[:], in_=cc_out[:])

            # ======== P5: finale ========
            gse = spool.tile([128, CO], f32, tag="gse")
            tmp_a = spool.tile([128, CO], f32, tag="tmpa")
            nc.vector.tensor_scalar_mul(gse[:], gst[:, 0:CO], bsel_t[:, 0:1])
            nc.vector.tensor_scalar_mul(tmp_a[:], gst[:, CO:2 * CO], bsel_t[:, 1:2])
            nc.vector.tensor_add(gse[:], gse[:], tmp_a[:])
            nc.vector.reciprocal(gse[:], gse[:])
            nc.vector.tensor_scalar_mul(gse[:], gse[:], gamma)
            cnt = 1.0 / (N_B * THW)
            mu = spool.tile([128, CO], f32, tag="mu")
            nc.vector.tensor_scalar_mul(mu[:], gst[:, 8:8 + CO], cnt)
            nc.vector.tensor_sub(mu[:], mu[:], bzc_t[:, 0:CO])
            ex2 = spool.tile([128, CO], f32, tag="ex2")
            nc.vector.tensor_scalar_mul(ex2[:], gst[:, 12:12 + CO], cnt)
            nc.vector.tensor_sub(ex2[:], ex2[:], bzc_t[:, CO:2 * CO])
            var = spool.tile([128, CO], f32, tag="var")
            nc.vector.tensor_mul(var[:], mu[:], mu[:])
            nc.vector.tensor_sub(var[:], ex2[:], var[:])
            nc.vector.tensor_scalar_add(var[:], var[:], BN_EPS)
            std = spool.tile([128, CO], f32, tag="std")
            nc.scalar.activation(std[:], var[:], SQRT)
            nc.vector.reciprocal(std[:], std[:])
            alpha = spool.tile([128, CO], f32, tag="alpha")
            nc.vector.tensor_mul(alpha[:], std[:], bnw_t[:])
            beta = spool.tile([128, CO], f32, tag="beta")
            nc.vector.tensor_mul(beta[:], mu[:], alpha[:])
            nc.vector.tensor_sub(beta[:], bnb_t[:], beta[:])

            p_out = tc.alloc_tile_pool(name="outp", bufs=2)
            for co in range(CO):
                mt = p_out.tile([128, T_LOC], f32, tag="mt")
                nc.vector.tensor_scalar_mul(mt[:], mt0[:, co, :], gse[:, co:co + 1])
                ot = p_out.tile([128, T_LOC], f32, tag="ot")
                nc.vector.tensor_scalar(ot[:], wy_t[:, co, :],
                                        alpha[:, co:co + 1], beta[:, co:co + 1],
                                        op0=MUL, op1=ADD)
                nc.vector.tensor_add(ot[:], ot[:], mt[:])
                nc.sync.dma_start(out=dview(out_loc)[:, co, :], in_=ot[:])
            p_out.release()
            p_mt0.release()
            p_expz.release()
            p_z.release()
            p_pmwy.release()

    nc.compile()
    return nc


def _prepare_maps(x, mask, Wh, bh, Wg, bg, Wm, bm, Wz, bz, bn_w, bn_b):
    import ml_dtypes

    xf = np.ascontiguousarray(x.reshape(N_B, C, THW), dtype=np.float32)
    mf = np.ascontiguousarray(mask.reshape(N_B, C, THW), dtype=np.float32)

    def chunked_bias(b):
        return np.ascontiguousarray(b.reshape(CO, 128).T, dtype=np.float32)

    wht = np.ascontiguousarray(Wh.T, dtype=np.float32)
    wgt = np.ascontiguousarray(Wg.T, dtype=np.float32)
    wmt = np.ascontiguousarray(Wm.T, dtype=np.float32)
    wzt = np.ascontiguousarray(Wz.T, dtype=np.float32)
    bh_row = np.broadcast_to(bh.astype(np.float32), (128, C)).copy()
    ones_bf = np.ones((128, 1), dtype=ml_dtypes.bfloat16)

    # BN bias compensation: raw sums include (8*T_LOC - N*THW) padded columns
    # where wy == bz exactly (x padded with zeros).
    n_pad = 8 * T_LOC - N_B * THW
    cntf = 1.0 / (N_B * THW)
    bzc = np.zeros((128, 8), np.float32)
    bzc[:, 0:4] = chunked_bias(bz * (n_pad * cntf))
    bzc[:, 4:8] = chunked_bias((bz * bz) * (n_pad * cntf))

    in_maps = []
    for core in range(8):
        n, q = divmod(core, 4)
        t0 = T_LOC * q
        valid = int(np.clip(THW - t0, 0, T_LOC))
        x_locc = np.zeros((C, T_LOC), np.float32)
        x_locc[:, :valid] = xf[n][:, t0:t0 + valid]
        x_fullc = np.zeros((C, S_PAD), np.float32)
        x_fullc[:, :THW] = xf[n]
        m_fullc = np.zeros((C, S_PAD), np.float32)
        m_fullc[:, :THW] = mf[n]
        # per-partition additive mask in [t-within-tile, t-tile] layout
        tmaddp = np.zeros((128, 16), np.float32)
        tgrid = (np.arange(NTT)[None, :] * 128 + np.arange(128)[:, None])
        tmaddp[:, :NTT] = np.where(tgrid < valid, 0.0, -1e30)
        bsel = np.zeros((128, 2), np.float32)
        bsel[:, 0] = 1.0 if n == 0 else 0.0
        bsel[:, 1] = 0.0 if n == 0 else 1.0
        in_maps.append(dict(
            x_full=x_fullc, mask_full=m_fullc, x_loc=x_locc,
            wht=wht, wgt=wgt, wmt=wmt, wzt=wzt,
            bh_in=chunked_bias(bh), bg_in=chunked_bias(bg),
            bm_in=chunked_bias(bm), bz_in=chunked_bias(bz),
            bh_row_in=bh_row,
            bnw_in=chunked_bias(bn_w), bnb_in=chunked_bias(bn_b),
            ones_in=ones_bf, tmaddp_in=tmaddp, bzc_in=bzc,
            bsel_in=bsel,
        ))
    return in_maps


def _estimate_shifts(xf, mf, Wh, bh, Wg, bg):
    # M1: safe global upper-bound estimate for the max of the energy matrix.
    # Any M1 in [true_max - 80, min_row_max + 85] keeps softmax exact
    # (constant shifts cancel); the window is tens wide so a sampled
    # estimate plus margin is bulletproof.
    ti = np.arange(0, THW, 41)
    si = np.arange(0, THW, 7)
    m_s = -np.inf
    for n in range(N_B):
        Q = (Wh @ xf[n][:, ti]) + bh[:, None]
        K = (Wg @ xf[n][:, si]) + bg[:, None]
        m_s = max(m_s, float((Q.T @ K).max()))
    m1 = m_s + 5.0
    # M2: norm bound on |ph_m| entries (second softmax argument is a convex
    # combination of ph_m values, so bounded by max |ph_m|).
    whn = float(np.linalg.norm(Wh, axis=1).max())
    mcn = max(float(np.linalg.norm(mf[n], axis=0).max()) for n in range(N_B))
    m2 = whn * mcn + float(np.abs(bh).max()) + 1.0
    return m1, m2


def kernel(x, mask, Wh, bh, Wg, bg, Wm, bm, Wz, bz, bn_w, bn_b, gamma,
           _debug=False, _trace=False):
    from concourse.bass_utils import run_bass_kernel_spmd

    x = np.asarray(x, np.float32)
    mask = np.asarray(mask, np.float32)
    Wh = np.asarray(Wh, np.float32); bh = np.asarray(bh, np.float32)
    Wg = np.asarray(Wg, np.float32); bg = np.asarray(bg, np.float32)
    Wm = np.asarray(Wm, np.float32); bm = np.asarray(bm, np.float32)
    Wz = np.asarray(Wz, np.float32); bz = np.asarray(bz, np.float32)
    bn_w = np.asarray(bn_w, np.float32); bn_b = np.asarray(bn_b, np.float32)
    gammaf = float(np.asarray(gamma))

    xf = x.reshape(N_B, C, THW)
    mf = mask.reshape(N_B, C, THW)
    m1, m2 = _estimate_shifts(xf, mf, Wh, bh, Wg, bg)
    key = (round(m1, 1), round(m2, 1), round(gammaf, 6), bool(_debug))
    if key not in _PROG_CACHE:
        _PROG_CACHE[key] = _build_program(key[0], key[1], gammaf, debug=_debug)
    nc = _PROG_CACHE[key]

    in_maps = _prepare_maps(x, mask, Wh, bh, Wg, bg, Wm, bm, Wz, bz, bn_w, bn_b)
    res = run_bass_kernel_spmd(nc, in_maps, core_ids=list(range(8)), trace=_trace)

    out = np.empty((N_B, C, THW), np.float32)
    for core in range(8):
        n, q = divmod(core, 4)
        t0 = T_LOC * q
        valid = int(np.clip(THW - t0, 0, T_LOC))
        if valid > 0:
            out[n][:, t0:t0 + valid] = res.results[core]["out_loc"][:, :valid]
    out = out.reshape(N_B, C, T, H, W)
    if _debug or _trace:
        return out, res
    return out



# revision 9
# speedup vs baseline: 1.0933x; 1.0933x over previous
"""Trainium2 Bass kernel for nn_SpaceTimeAtten (space-time attention block).

Contract: kernel(**inputs) takes FULL unsharded numpy inputs (see reference
setup_inputs) and returns the FULL (2, 512, 8, 28, 28) float32 output.

Sharding: 8 cores = 2 batches x 4 query-chunks (t = local THW quarter).

Per-core structure (v2):
  - All projections in bf16 (inputs/weights pre-cast on host). wy conv runs
    first; its BN partial sums go out in an early 8-core AllReduce (AR1) that
    completes while attention runs.
  - K-side pg and V-side phm are computed for the FULL s range in one piece
    loop; pg (and the Q-side phx) are stored as fp8e4 so both operands of the
    energy matmul run in DoubleRow perf mode (2 k-rows per PE pass).
  - Attention is a single pass over all 49 s-tiles, 4 query blocks of up to
    512 t. The energy is built TRANSPOSED (E^T = [s_part, t_free]) so
    exp(E^T - M1) is directly the rhs of the PV matmul with phm chunks as
    stationary -> PV output lands in [c_part, t_free] layout, no transposes.
  - Row sums r_t come from a ones-vector matmul accumulated in PSUM. 1/r is
    broadcast across partitions with a rank-1 PE matmul and applied by DVE.
  - Second softmax denominators (per channel, over t) need a 4-core reduce
    per batch: one [128,4] AllReduce with groups [[0..3],[4..7]] (AR2) at the
    end; only a 2-op fuse + output DMA depend on it.
"""

import os
import numpy as np

# ---- problem constants (hardcoded per contract) ----
N_B, C, T, H, W = 2, 512, 8, 28, 28
THW = T * H * W            # 6272
BN_EPS = 1e-5

CI = 4                     # 128-chunks of the channel dim
CO = 4
S_PAD = 6272               # 49 s-tiles of 128 (exact, no padding)
NST = 49
T_LOC = 1664               # local t per core (13 tiles of 128)
NTT = 13
TBLOCKS = [(0, 512), (512, 512), (1024, 512), (1536, 128)]  # (t0, tfree)
R_EPS = 1e-30

_PROG_CACHE = {}


def _build_program(m1, m2, gamma, use_fp8=True):
    import concourse.bass as bass
    import concourse.mybir as mybir
    import concourse.tile as tile
    from concourse import bacc

    f32 = mybir.dt.float32
    bf16 = mybir.dt.bfloat16
    fp8 = mybir.dt.float8e4
    qk_dt = fp8 if use_fp8 else bf16
    EXP = mybir.ActivationFunctionType.Exp
    SQRT = mybir.ActivationFunctionType.Sqrt
    SQUARE = mybir.ActivationFunctionType.Square
    IDENT = mybir.ActivationFunctionType.Identity
    DR = mybir.MatmulPerfMode.DoubleRow
    AX = mybir.AxisListType.X
    MUL = mybir.AluOpType.mult
    ADD = mybir.AluOpType.add

    FC = T_LOC // 4  # 416

    nc = bacc.Bacc("TRN2")

    x_full = nc.dram_tensor("x_full", [C, S_PAD], bf16, kind="ExternalInput")
    mask_full = nc.dram_tensor("mask_full", [C, S_PAD], bf16, kind="ExternalInput")
    x_loc = nc.dram_tensor("x_loc", [C, T_LOC], bf16, kind="ExternalInput")
    wht = nc.dram_tensor("wht", [C, C], bf16, kind="ExternalInput")
    wgt = nc.dram_tensor("wgt", [C, C], bf16, kind="ExternalInput")
    wmt = nc.dram_tensor("wmt", [C, C], bf16, kind="ExternalInput")
    wzt = nc.dram_tensor("wzt", [C, C], bf16, kind="ExternalInput")
    bh_in = nc.dram_tensor("bh_in", [128, CO], f32, kind="ExternalInput")
    bg_in = nc.dram_tensor("bg_in", [128, CO], f32, kind="ExternalInput")
    bm_in = nc.dram_tensor("bm_in", [128, CO], f32, kind="ExternalInput")
    bz_in = nc.dram_tensor("bz_in", [128, CO], f32, kind="ExternalInput")
    bh_row_in = nc.dram_tensor("bh_row_in", [128, C], f32, kind="ExternalInput")
    bnw_in = nc.dram_tensor("bnw_in", [128, CO], f32, kind="ExternalInput")
    bnb_in = nc.dram_tensor("bnb_in", [128, CO], f32, kind="ExternalInput")
    ones_in = nc.dram_tensor("ones_in", [128, 1], bf16, kind="ExternalInput")
    onesrow_in = nc.dram_tensor("onesrow_in", [1, 128], bf16, kind="ExternalInput")
    hmask_in = nc.dram_tensor("hmask_in", [1, T_LOC], f32, kind="ExternalInput")
    secorr_in = nc.dram_tensor("secorr_in", [128, CO], f32, kind="ExternalInput")
    bzc_in = nc.dram_tensor("bzc_in", [128, 8], f32, kind="ExternalInput")

    out_loc = nc.dram_tensor("out_loc", [C, T_LOC], f32, kind="ExternalOutput")

    cc1_in = nc.dram_tensor("cc1_in", [128, 8], f32)
    cc1_out = nc.dram_tensor("cc1_out", [128, 8], f32)
    cc2_in = nc.dram_tensor("cc2_in", [128, CO], f32)
    cc2_out = nc.dram_tensor("cc2_out", [128, CO], f32)

    def dview(dram):
        return dram.rearrange("(k p) s -> p k s", p=128)

    with tile.TileContext(nc) as tc:
        with (
            tc.tile_pool(name="const", bufs=1) as cpool,
            tc.tile_pool(name="small", bufs=1) as spool,
        ):
            # ---- constants (gpsimd queue) ----
            ones_t = cpool.tile([128, 1], bf16, tag="ones")
            onesrow_t = cpool.tile([1, 128], bf16, tag="onesrow")
            bh_t = cpool.tile([128, CO], f32, tag="bh")
            bg_t = cpool.tile([128, CO], f32, tag="bg")
            bm_t = cpool.tile([128, CO], f32, tag="bm")
            bz_t = cpool.tile([128, CO], f32, tag="bz")
            bnw_t = cpool.tile([128, CO], f32, tag="bnw")
            bnb_t = cpool.tile([128, CO], f32, tag="bnb")
            bzc_t = cpool.tile([128, 8], f32, tag="bzc")
            secorr_t = cpool.tile([128, CO], f32, tag="secorr")
            hmask_t = cpool.tile([1, T_LOC], f32, tag="hmask")
            bh_row = cpool.tile([128, C], f32, tag="bhrow")
            for tl, dr in ((ones_t, ones_in), (onesrow_t, onesrow_in),
                           (bh_t, bh_in), (bg_t, bg_in), (bm_t, bm_in),
                           (bz_t, bz_in), (bnw_t, bnw_in), (bnb_t, bnb_in),
                           (bzc_t, bzc_in), (secorr_t, secorr_in),
                           (hmask_t, hmask_in), (bh_row, bh_row_in)):
                nc.gpsimd.dma_start(out=tl[:], in_=dr[:])
            m1b = cpool.tile([128, 1], f32, tag="m1b")
            nc.vector.memset(m1b[:], -m1)
            m2b = cpool.tile([128, 1], f32, tag="m2b")
            nc.vector.memset(m2b[:], -m2)

            # ---- persistent tiles (allocated first: released last) ----
            p_phx = tc.alloc_tile_pool(name="phxp", bufs=1)
            phx = p_phx.tile([128, CI, T_LOC], qk_dt, tag="phx")
            p_kv = tc.alloc_tile_pool(name="kvp", bufs=1)
            pgh = p_kv.tile([128, CI, S_PAD], qk_dt, tag="pgh")
            phmh = p_kv.tile([128, NST, C], bf16, tag="phmh")
            p_wyp = tc.alloc_tile_pool(name="wypp", bufs=1, side="right")
            wy_bf = p_wyp.tile([128, CO, T_LOC], f32, tag="wy")
            pm_bf = p_wyp.tile([128, CO, T_LOC], bf16, tag="pm")

            # ---- weights (gpsimd queue; wz first for the early wy conv) ----
            p_w = tc.alloc_tile_pool(name="wp", bufs=1)
            wt_z = p_w.tile([128, CI, C], bf16, tag="wz")
            wt_h = p_w.tile([128, CI, C], bf16, tag="wh")
            wt_g = p_w.tile([128, CI, C], bf16, tag="wg")
            wt_m = p_w.tile([128, CI, C], bf16, tag="wm")
            nc.gpsimd.dma_start(out=wt_z[:], in_=dview(wzt))
            nc.gpsimd.dma_start(out=wt_h[:], in_=dview(wht))
            nc.gpsimd.dma_start(out=wt_g[:], in_=dview(wgt))
            nc.gpsimd.dma_start(out=wt_m[:], in_=dview(wmt))

            # local x (sync queue, first so it wins the queue at start)
            p_xl = tc.alloc_tile_pool(name="xlp", bufs=1)
            xloc_t = p_xl.tile([128, CI, T_LOC], bf16, tag="xloc")
            nc.sync.dma_start(out=xloc_t[:], in_=dview(x_loc))

            stats1 = spool.tile([128, 8], f32, tag="stats1")
            se_parts = spool.tile([128, 16], f32, tag="separts")
            se_loc = spool.tile([128, CO], f32, tag="seloc")

            ps_cv = tc.alloc_tile_pool(name="pscv", bufs=2, space="PSUM")
            p_scr = tc.alloc_tile_pool(name="scrp", bufs=2)

            # ======== wy conv (bf16) + BN partials + AR1 ========
            for co in range(CO):
                for fc in range(4):
                    ps = ps_cv.tile([128, 512], f32, tag="c")
                    for ci in range(CI):
                        nc.tensor.matmul(
                            ps[:, :FC],
                            wt_z[:, ci, co * 128:(co + 1) * 128],
                            xloc_t[:, ci, fc * FC:(fc + 1) * FC],
                            start=(ci == 0), stop=(ci == CI - 1))
                    nc.scalar.activation(
                        wy_bf[:, co, fc * FC:(fc + 1) * FC], ps[:, :FC],
                        IDENT, bias=bz_t[:, co:co + 1], scale=1.0)
                nc.vector.reduce_sum(stats1[:, co:co + 1], wy_bf[:, co, :],
                                     axis=AX)
                scr = p_scr.tile([128, T_LOC], bf16, tag="scr")
                nc.scalar.activation(scr[:], wy_bf[:, co, :], SQUARE,
                                     accum_out=stats1[:, 4 + co:5 + co])
            nc.sync.dma_start(out=cc1_in[:], in_=stats1[:])
            nc.gpsimd.collective_compute(
                "AllReduce", ADD,
                replica_groups=[[0, 1, 2, 3, 4, 5, 6, 7]],
                ins=[cc1_in[:]], outs=[cc1_out[:]])

            # ======== K/V conv piece loop (x on sync, mask on vector) ========
            p_piece = tc.alloc_tile_pool(name="piecep", bufs=2)
            o = 0
            pieces = []
            while o < NST:
                w = min(4, NST - o)
                pieces.append((o, w))
                o += w
            for (pt0, ptw) in pieces:
                s_off = pt0 * 128
                pw = ptw * 128
                xp = p_piece.tile([128, CI, 512], bf16, tag="xp", name="xp")
                nc.sync.dma_start(
                    out=xp[:, :, :pw],
                    in_=dview(x_full)[:, :, s_off:s_off + pw])
                for co in range(CO):
                    ps = ps_cv.tile([128, 512], f32, tag="c")
                    for ci in range(CI):
                        nc.tensor.matmul(
                            ps[:, :pw],
                            wt_g[:, ci, co * 128:(co + 1) * 128],
                            xp[:, ci, :pw],
                            start=(ci == 0), stop=(ci == CI - 1))
                    nc.scalar.activation(
                        pgh[:, co, s_off:s_off + pw], ps[:, :pw],
                        IDENT, bias=bg_t[:, co:co + 1], scale=1.0)
                mp = p_piece.tile([128, CI, 512], bf16, tag="mp", name="mp")
                nc.scalar.dma_start(
                    out=mp[:, :, :pw],
                    in_=dview(mask_full)[:, :, s_off:s_off + pw])
                for sj in range(ptw):
                    st = pt0 + sj
                    ps = ps_cv.tile([128, 512], f32, tag="c")
                    for ci in range(CI):
                        nc.tensor.matmul(
                            ps[:],
                            mp[:, ci, sj * 128:(sj + 1) * 128],
                            wt_h[:, ci, :],
                            start=(ci == 0), stop=(ci == CI - 1))
                    nc.vector.tensor_add(phmh[:, st, :], ps[:], bh_row[:])

            # ======== Q conv (fp8/bf16 out) + pm conv (bf16 out) ========
            for co in range(CO):
                for fc in range(4):
                    ps = ps_cv.tile([128, 512], f32, tag="c")
                    for ci in range(CI):
                        nc.tensor.matmul(
                            ps[:, :FC],
                            wt_h[:, ci, co * 128:(co + 1) * 128],
                            xloc_t[:, ci, fc * FC:(fc + 1) * FC],
                            start=(ci == 0), stop=(ci == CI - 1))
                    nc.scalar.activation(
                        phx[:, co, fc * FC:(fc + 1) * FC], ps[:, :FC],
                        IDENT, bias=bh_t[:, co:co + 1], scale=1.0)
            for co in range(CO):
                for fc in range(4):
                    ps = ps_cv.tile([128, 512], f32, tag="c")
                    for ci in range(CI):
                        nc.tensor.matmul(
                            ps[:, :FC],
                            wt_m[:, ci, co * 128:(co + 1) * 128],
                            xloc_t[:, ci, fc * FC:(fc + 1) * FC],
                            start=(ci == 0), stop=(ci == CI - 1))
                    nc.scalar.activation(
                        pm_bf[:, co, fc * FC:(fc + 1) * FC], ps[:, :FC],
                        IDENT, bias=bm_t[:, co:co + 1], scale=1.0)

            p_piece.release()
            p_scr.release()
            ps_cv.release()
            p_xl.release()
            p_w.release()

            # ======== attention: 4 t-blocks x 49 s-tiles, one pass ========
            p_expz = tc.alloc_tile_pool(name="expzp", bufs=1)
            expz = p_expz.tile([128, CO, T_LOC], bf16, tag="expz")
            ps_att = tc.alloc_tile_pool(name="psatt", bufs=1, space="PSUM")
            p_pt = tc.alloc_tile_pool(name="ptp", bufs=3)
            p_z = tc.alloc_tile_pool(name="zp", bufs=2)
            p_rb = tc.alloc_tile_pool(name="rbp", bufs=2)
            p_rr = tc.alloc_tile_pool(name="rrp", bufs=2)

            for bi, (t0, tfree) in enumerate(TBLOCKS):
                ocs = [ps_att.tile([128, 512], f32, tag=f"o{j}",
                                   name=f"o{j}_{bi}") for j in range(CO)]
                rps = ps_att.tile([1, 512], f32, tag="r", name=f"r{bi}")

                def emit_qk(st):
                    eps_t = ps_att.tile([128, 512], f32, tag="e", bufs=2,
                                        name=f"e{bi}_{st}")
                    if use_fp8:
                        for p2 in range(0, CI, 2):
                            nc.tensor.matmul(
                                eps_t[:, :tfree],
                                pgh[:, p2:p2 + 2, st * 128:(st + 1) * 128],
                                phx[:, p2:p2 + 2, t0:t0 + tfree],
                                start=(p2 == 0), stop=(p2 == CI - 2),
                                perf_mode=DR)
                    else:
                        for ci in range(CI):
                            nc.tensor.matmul(
                                eps_t[:, :tfree],
                                pgh[:, ci, st * 128:(st + 1) * 128],
                                phx[:, ci, t0:t0 + tfree],
                                start=(ci == 0), stop=(ci == CI - 1))
                    ptile = p_pt.tile([128, 512], bf16, tag="pt",
                                      name=f"pt{bi}_{st}")
                    nc.scalar.activation(ptile[:, :tfree], eps_t[:, :tfree],
                                         EXP, bias=m1b[:], scale=1.0)
                    return ptile

                nxt = emit_qk(0)
                for st in range(NST):
                    ptile = nxt
                    if st + 1 < NST:
                        nxt = emit_qk(st + 1)
                    for co in range(CO):
                        nc.tensor.matmul(
                            ocs[co][:, :tfree],
                            phmh[:, st, co * 128:(co + 1) * 128],
                            ptile[:, :tfree],
                            start=(st == 0), stop=(st == NST - 1))
                    nc.tensor.matmul(
                        rps[:, :tfree],
                        ones_t[:],
                        ptile[:, :tfree],
                        start=(st == 0), stop=(st == NST - 1))

                # block epilogue: rb = 1/(r + hmask) broadcast, z, exp, se
                rrow = p_rr.tile([1, 512], f32, tag="rrow", name=f"rrow{bi}")
                nc.vector.tensor_add(rrow[0:1, :tfree], rps[0:1, :tfree],
                                     hmask_t[0:1, t0:t0 + tfree])
                rrbf = p_rr.tile([1, 512], bf16, tag="rrbf", name=f"rrbf{bi}")
                with nc.allow_low_precision(reason="1/r broadcast in bf16"):
                    nc.vector.reciprocal(rrbf[0:1, :tfree], rrow[0:1, :tfree])
                rbb = ps_att.tile([128, 512], f32, tag="rbb", name=f"rbb{bi}")
                nc.tensor.matmul(rbb[:, :tfree], onesrow_t[0:1, :],
                                 rrbf[0:1, :tfree], start=True, stop=True)
                rb_sb = p_rb.tile([128, 512], bf16, tag="rb", name=f"rb{bi}")
                nc.vector.tensor_copy(rb_sb[:, :tfree], rbb[:, :tfree])
                for co in range(CO):
                    z_sb = p_z.tile([128, 512], f32, tag="z",
                                    name=f"z{bi}_{co}")
                    nc.vector.tensor_mul(z_sb[:, :tfree], ocs[co][:, :tfree],
                                         rb_sb[:, :tfree])
                    nc.scalar.activation(
                        expz[:, co, t0:t0 + tfree], z_sb[:, :tfree],
                        EXP, bias=m2b[:], scale=1.0,
                        accum_out=se_parts[:, co * 4 + bi:co * 4 + bi + 1])

                if bi == 0:
                    # AR1 landed long ago: BN finalization + wyfin, off the
                    # critical path (vector ops amid attention).
                    cnt = 1.0 / (N_B * THW)
                    gst1 = spool.tile([128, 8], f32, tag="gst1")
                    nc.scalar.dma_start(out=gst1[:], in_=cc1_out[:])
                    mu = spool.tile([128, CO], f32, tag="mu")
                    nc.vector.tensor_scalar_mul(mu[:], gst1[:, 0:CO], cnt)
                    nc.vector.tensor_sub(mu[:], mu[:], bzc_t[:, 0:CO])
                    ex2 = spool.tile([128, CO], f32, tag="ex2")
                    nc.vector.tensor_scalar_mul(ex2[:], gst1[:, CO:2 * CO],
                                                cnt)
                    nc.vector.tensor_sub(ex2[:], ex2[:], bzc_t[:, CO:2 * CO])
                    var = spool.tile([128, CO], f32, tag="var")
                    nc.vector.tensor_mul(var[:], mu[:], mu[:])
                    nc.vector.tensor_sub(var[:], ex2[:], var[:])
                    nc.vector.tensor_scalar_add(var[:], var[:], BN_EPS)
                    std = spool.tile([128, CO], f32, tag="std")
                    nc.scalar.activation(std[:], var[:], SQRT)
                    alpha = spool.tile([128, CO], f32, tag="alpha")
                    nc.vector.reciprocal(alpha[:], std[:])
                    nc.vector.tensor_mul(alpha[:], alpha[:], bnw_t[:])
                    beta = spool.tile([128, CO], f32, tag="beta")
                    nc.vector.tensor_mul(beta[:], mu[:], alpha[:])
                    nc.vector.tensor_sub(beta[:], bnb_t[:], beta[:])
                    for co in range(CO):
                        nc.vector.tensor_scalar(
                            wy_bf[:, co, :], wy_bf[:, co, :],
                            alpha[:, co:co + 1], beta[:, co:co + 1],
                            op0=MUL, op1=ADD)

            p_rr.release()
            p_rb.release()
            p_z.release()
            p_pt.release()
            ps_att.release()

            # ======== tail: AR2 + fuse ========
            for co in range(CO):
                nc.vector.reduce_sum(se_loc[:, co:co + 1],
                                     se_parts[:, co * 4:(co + 1) * 4], axis=AX)
            nc.vector.tensor_sub(se_loc[:], se_loc[:], secorr_t[:])
            nc.sync.dma_start(out=cc2_in[:], in_=se_loc[:])
            nc.gpsimd.collective_compute(
                "AllReduce", ADD,
                replica_groups=[[0, 1, 2, 3], [4, 5, 6, 7]],
                ins=[cc2_in[:]], outs=[cc2_out[:]])

            # mt0 = expz * pm, independent of AR2 (fills the wait)
            for co in range(CO):
                nc.vector.tensor_mul(expz[:, co, :], expz[:, co, :],
                                     pm_bf[:, co, :])

            gst2 = spool.tile([128, CO], f32, tag="gst2")
            nc.scalar.dma_start(out=gst2[:], in_=cc2_out[:])
            gse = spool.tile([128, CO], f32, tag="gse")
            nc.vector.reciprocal(gse[:], gst2[:])
            nc.vector.tensor_scalar_mul(gse[:], gse[:], gamma)

            p_out = tc.alloc_tile_pool(name="outp", bufs=2)
            for co in range(CO):
                ot = p_out.tile([128, T_LOC], f32, tag="ot")
                nc.vector.tensor_scalar_mul(ot[:], expz[:, co, :],
                                            gse[:, co:co + 1])
                nc.vector.tensor_add(ot[:], ot[:], wy_bf[:, co, :])
                nc.sync.dma_start(out=dview(out_loc)[:, co, :], in_=ot[:])
            p_out.release()
            p_expz.release()
            p_wyp.release()
            p_kv.release()
            p_phx.release()

    nc.compile()
    return nc


def _prepare_maps(x, mask, Wh, bh, Wg, bg, Wm, bm, Wz, bz, bn_w, bn_b, m2r):
    import ml_dtypes
    bf16 = ml_dtypes.bfloat16

    xf = np.ascontiguousarray(x.reshape(N_B, C, THW), dtype=np.float32)
    mf = np.ascontiguousarray(mask.reshape(N_B, C, THW), dtype=np.float32)

    def chunked_bias(b):
        return np.ascontiguousarray(b.reshape(CO, 128).T, dtype=np.float32)

    wht = np.ascontiguousarray(Wh.T).astype(bf16)
    wgt = np.ascontiguousarray(Wg.T).astype(bf16)
    wmt = np.ascontiguousarray(Wm.T).astype(bf16)
    wzt = np.ascontiguousarray(Wz.T).astype(bf16)
    bh_row = np.broadcast_to(bh.astype(np.float32), (128, C)).copy()
    ones_bf = np.ones((128, 1), dtype=bf16)
    onesrow_bf = np.ones((1, 128), dtype=bf16)

    # BN bias compensation: raw sums include (8*T_LOC - N*THW) padded columns
    # where wy == bz exactly (x padded with zeros).
    n_pad = 8 * T_LOC - N_B * THW
    cntf = 1.0 / (N_B * THW)
    bzc = np.zeros((128, 8), np.float32)
    bzc[:, 0:4] = chunked_bias(bz * (n_pad * cntf))
    bzc[:, 4:8] = chunked_bias((bz * bz) * (n_pad * cntf))

    in_maps = []
    for core in range(8):
        n, q = divmod(core, 4)
        t0 = T_LOC * q
        valid = int(np.clip(THW - t0, 0, T_LOC))
        x_locc = np.zeros((C, T_LOC), bf16)
        x_locc[:, :valid] = xf[n][:, t0:t0 + valid].astype(bf16)
        # hmask: tiny eps on valid t, huge on padded t so rb = 1/(r+hmask) ~ 0
        hmask = np.full((1, T_LOC), 1e30, np.float32)
        hmask[0, :valid] = R_EPS
        # padded t columns contribute exp(0 - m2) each to the se sums
        secorr = np.full((128, CO), (T_LOC - valid) * np.exp(-m2r), np.float32)
        in_maps.append(dict(
            x_full=xf[n].astype(bf16), mask_full=mf[n].astype(bf16),
            x_loc=x_locc,
            wht=wht, wgt=wgt, wmt=wmt, wzt=wzt,
            bh_in=chunked_bias(bh), bg_in=chunked_bias(bg),
            bm_in=chunked_bias(bm), bz_in=chunked_bias(bz),
            bh_row_in=bh_row,
            bnw_in=chunked_bias(bn_w), bnb_in=chunked_bias(bn_b),
            ones_in=ones_bf, onesrow_in=onesrow_bf,
            hmask_in=hmask, secorr_in=secorr, bzc_in=bzc,
        ))
    return in_maps


def _estimate_shifts(xf, mf, Wh, bh, Wg, bg):
    # M1: safe global upper-bound estimate for the max of the energy matrix.
    # Any constant shift cancels exactly in softmax; the +5 margin absorbs
    # sampling misses and fp8 quantization noise.
    ti = np.arange(0, THW, 41)
    si = np.arange(0, THW, 7)
    m_s = -np.inf
    for n in range(N_B):
        Q = (Wh @ xf[n][:, ti]) + bh[:, None]
        K = (Wg @ xf[n][:, si]) + bg[:, None]
        m_s = max(m_s, float((Q.T @ K).max()))
    m1 = m_s + 5.0
    # M2: norm bound on |ph_m| entries (second softmax argument is a convex
    # combination of ph_m values, so bounded by max |ph_m|).
    whn = float(np.linalg.norm(Wh, axis=1).max())
    mcn = max(float(np.linalg.norm(mf[n], axis=0).max()) for n in range(N_B))
    m2 = whn * mcn + float(np.abs(bh).max()) + 1.0
    return m1, m2


def kernel(x, mask, Wh, bh, Wg, bg, Wm, bm, Wz, bz, bn_w, bn_b, gamma,
           _debug=False, _trace=False):
    from concourse.bass_utils import run_bass_kernel_spmd

    x = np.asarray(x, np.float32)
    mask = np.asarray(mask, np.float32)
    Wh = np.asarray(Wh, np.float32); bh = np.asarray(bh, np.float32)
    Wg = np.asarray(Wg, np.float32); bg = np.asarray(bg, np.float32)
    Wm = np.asarray(Wm, np.float32); bm = np.asarray(bm, np.float32)
    Wz = np.asarray(Wz, np.float32); bz = np.asarray(bz, np.float32)
    bn_w = np.asarray(bn_w, np.float32); bn_b = np.asarray(bn_b, np.float32)
    gammaf = float(np.asarray(gamma))
    use_fp8 = os.environ.get("BASS_NO_FP8", "0") != "1"

    xf = x.reshape(N_B, C, THW)
    mf = mask.reshape(N_B, C, THW)
    m1, m2 = _estimate_shifts(xf, mf, Wh, bh, Wg, bg)
    key = (round(m1, 1), round(m2, 1), round(gammaf, 6), use_fp8)
    if key not in _PROG_CACHE:
        _PROG_CACHE[key] = _build_program(key[0], key[1], gammaf,
                                          use_fp8=use_fp8)
    nc = _PROG_CACHE[key]

    in_maps = _prepare_maps(x, mask, Wh, bh, Wg, bg, Wm, bm, Wz, bz,
                            bn_w, bn_b, key[1])
    res = run_bass_kernel_spmd(nc, in_maps, core_ids=list(range(8)),
                               trace=_trace)

    out = np.empty((N_B, C, THW), np.float32)
    for core in range(8):
        n, q = divmod(core, 4)
        t0 = T_LOC * q
        valid = int(np.clip(THW - t0, 0, T_LOC))
        if valid > 0:
            out[n][:, t0:t0 + valid] = res.results[core]["out_loc"][:, :valid]
    out = out.reshape(N_B, C, T, H, W)
    if _debug or _trace:
        return out, res
    return out


# revision 20
# speedup vs baseline: 1.4333x; 1.3110x over previous
"""Trainium2 Bass kernel for nn_SpaceTimeAtten (space-time attention block).

Contract: kernel(**inputs) takes FULL unsharded numpy inputs (see reference
setup_inputs) and returns the FULL (2, 512, 8, 28, 28) float32 output.

Sharding: 8 cores = 2 batches x 4 query-chunks (t = local THW quarter).

Per-core structure (v2):
  - All projections in bf16 (inputs/weights pre-cast on host). wy conv runs
    first; its BN partial sums go out in an early 8-core AllReduce (AR1) that
    completes while attention runs.
  - K-side pg and V-side phm are computed for the FULL s range in one piece
    loop; pg (and the Q-side phx) are stored as fp8e4 so both operands of the
    energy matmul run in DoubleRow perf mode (2 k-rows per PE pass).
  - Attention is a single pass over all 49 s-tiles, 4 query blocks of up to
    512 t. The energy is built TRANSPOSED (E^T = [s_part, t_free]) so
    exp(E^T - M1) is directly the rhs of the PV matmul with phm chunks as
    stationary -> PV output lands in [c_part, t_free] layout, no transposes.
  - Row sums r_t come from a ones-vector matmul accumulated in PSUM. 1/r is
    broadcast across partitions with a rank-1 PE matmul and applied by DVE.
  - Second softmax denominators (per channel, over t) need a 4-core reduce
    per batch: one [128,4] AllReduce with groups [[0..3],[4..7]] (AR2) at the
    end; only a 2-op fuse + output DMA depend on it.
"""

import os
import numpy as np

# ---- problem constants (hardcoded per contract) ----
N_B, C, T, H, W = 2, 512, 8, 28, 28
THW = T * H * W            # 6272
BN_EPS = 1e-5

CI = 4                     # 128-chunks of the channel dim
CO = 4
S_PAD = 6272               # 49 s-tiles of 128 (exact, no padding)
NST = 49
T_LOC = 1664               # local t per core (13 tiles of 128)
NTT = 13
TBLOCKS = [(0, 416), (416, 416), (832, 416), (1248, 416)]  # (t0, tfree)
R_EPS = 1e-30

_PROG_CACHE = {}


def _build_program(m1, m2, gamma, use_fp8=True):
    import concourse.bass as bass
    import concourse.mybir as mybir
    import concourse.tile as tile
    from concourse import bacc

    f32 = mybir.dt.float32
    bf16 = mybir.dt.bfloat16
    fp8 = mybir.dt.float8e4
    qk_dt = fp8 if use_fp8 else bf16
    EXP = mybir.ActivationFunctionType.Exp
    SQRT = mybir.ActivationFunctionType.Sqrt
    SQUARE = mybir.ActivationFunctionType.Square
    IDENT = mybir.ActivationFunctionType.Identity
    DR = mybir.MatmulPerfMode.DoubleRow
    AX = mybir.AxisListType.X
    MUL = mybir.AluOpType.mult
    ADD = mybir.AluOpType.add

    FC = T_LOC // 4  # 416

    nc = bacc.Bacc("TRN2")

    x_full = nc.dram_tensor("x_full", [C, S_PAD], bf16, kind="ExternalInput")
    mask_full = nc.dram_tensor("mask_full", [C, S_PAD], bf16, kind="ExternalInput")
    x_loc = nc.dram_tensor("x_loc", [C, T_LOC], bf16, kind="ExternalInput")
    wht = nc.dram_tensor("wht", [C, C], bf16, kind="ExternalInput")
    wgt = nc.dram_tensor("wgt", [C, C], bf16, kind="ExternalInput")
    wmt = nc.dram_tensor("wmt", [C, C], bf16, kind="ExternalInput")
    wzt = nc.dram_tensor("wzt", [C, C], bf16, kind="ExternalInput")
    bh_in = nc.dram_tensor("bh_in", [128, CO], f32, kind="ExternalInput")
    bg_in = nc.dram_tensor("bg_in", [128, CO], f32, kind="ExternalInput")
    bm_in = nc.dram_tensor("bm_in", [128, CO], f32, kind="ExternalInput")
    bz_in = nc.dram_tensor("bz_in", [128, CO], f32, kind="ExternalInput")
    bh_row_in = nc.dram_tensor("bh_row_in", [128, C], f32, kind="ExternalInput")
    bnw_in = nc.dram_tensor("bnw_in", [128, CO], f32, kind="ExternalInput")
    bnb_in = nc.dram_tensor("bnb_in", [128, CO], f32, kind="ExternalInput")
    ones_in = nc.dram_tensor("ones_in", [128, 1], bf16, kind="ExternalInput")
    onesrow_in = nc.dram_tensor("onesrow_in", [1, 128], bf16, kind="ExternalInput")
    hmask_in = nc.dram_tensor("hmask_in", [1, T_LOC], f32, kind="ExternalInput")
    secorr_in = nc.dram_tensor("secorr_in", [128, CO], f32, kind="ExternalInput")
    bzc_in = nc.dram_tensor("bzc_in", [128, 8], f32, kind="ExternalInput")
    rstd0_in = nc.dram_tensor("rstd0_in", [128, CO], f32, kind="ExternalInput")

    out_loc = nc.dram_tensor("out_loc", [C, T_LOC], bf16,
                             kind="ExternalOutput")

    cc1_in = nc.dram_tensor("cc1_in", [128, 8], f32)
    cc1_out = nc.dram_tensor("cc1_out", [128, 8], f32)
    cc2_in = nc.dram_tensor("cc2_in", [128, CO], f32)
    cc2_out = nc.dram_tensor("cc2_out", [128, CO], f32)

    def dview(dram):
        return dram.rearrange("(k p) s -> p k s", p=128)

    with tile.TileContext(nc) as tc:
        with (
            tc.tile_pool(name="const", bufs=1) as cpool,
            tc.tile_pool(name="small", bufs=1) as spool,
        ):
            # ---- constants (gpsimd queue) ----
            ones_t = cpool.tile([128, 1], bf16, tag="ones")
            onesrow_t = cpool.tile([1, 128], bf16, tag="onesrow")
            bh_t = cpool.tile([128, CO], f32, tag="bh")
            bg_t = cpool.tile([128, CO], f32, tag="bg")
            bm_t = cpool.tile([128, CO], f32, tag="bm")
            bz_t = cpool.tile([128, CO], f32, tag="bz")
            bnw_t = cpool.tile([128, CO], f32, tag="bnw")
            bnb_t = cpool.tile([128, CO], f32, tag="bnb")
            bzc_t = cpool.tile([128, 8], f32, tag="bzc")
            secorr_t = cpool.tile([128, CO], f32, tag="secorr")
            hmask_t = cpool.tile([1, T_LOC], f32, tag="hmask")
            bh_row = cpool.tile([128, C], f32, tag="bhrow")
            rstd0_t = cpool.tile([128, CO], f32, tag="rstd0")
            for tl, dr in ((ones_t, ones_in), (onesrow_t, onesrow_in),
                           (bh_t, bh_in), (bg_t, bg_in), (bm_t, bm_in),
                           (bz_t, bz_in), (bnw_t, bnw_in), (bnb_t, bnb_in),
                           (bzc_t, bzc_in), (secorr_t, secorr_in),
                           (hmask_t, hmask_in), (bh_row, bh_row_in),
                           (rstd0_t, rstd0_in)):
                nc.gpsimd.dma_start(out=tl[:], in_=dr[:])
            m1b = cpool.tile([128, 1], f32, tag="m1b")
            nc.vector.memset(m1b[:], -m1)
            m2b = cpool.tile([128, 1], f32, tag="m2b")
            nc.vector.memset(m2b[:], -m2)

            # ---- persistent tiles (allocated first: released last) ----
            p_phx = tc.alloc_tile_pool(name="phxp", bufs=1)
            phx = p_phx.tile([128, CI, T_LOC], qk_dt, tag="phx")
            p_kv = tc.alloc_tile_pool(name="kvp", bufs=1)
            pgh = p_kv.tile([128, CI, S_PAD], qk_dt, tag="pgh")
            phmh = p_kv.tile([128, NST, C], bf16, tag="phmh")
            p_wyp = tc.alloc_tile_pool(name="wypp", bufs=1, side="right")
            wy_bf = p_wyp.tile([128, CO, T_LOC], f32, tag="wy")
            pm_bf = p_wyp.tile([128, CO, T_LOC], bf16, tag="pm")

            # ---- weights (gpsimd queue; wz first for the early wy conv) ----
            p_w = tc.alloc_tile_pool(name="wp", bufs=1)
            wt_z = p_w.tile([128, CI, C], bf16, tag="wz")
            wt_h = p_w.tile([128, CI, C], bf16, tag="wh")
            wt_g = p_w.tile([128, CI, C], bf16, tag="wg")
            wt_m = p_w.tile([128, CI, C], bf16, tag="wm")
            nc.gpsimd.dma_start(out=wt_z[:], in_=dview(wzt))
            nc.gpsimd.dma_start(out=wt_h[:], in_=dview(wht))
            nc.gpsimd.dma_start(out=wt_g[:], in_=dview(wgt))
            nc.gpsimd.dma_start(out=wt_m[:], in_=dview(wmt))

            # local x (sync queue, first, chunked so the wy conv starts early)
            p_xl = tc.alloc_tile_pool(name="xlp", bufs=1)
            xloc_t = p_xl.tile([128, CI, T_LOC], bf16, tag="xloc")
            for fc in range(4):
                nc.sync.dma_start(
                    out=xloc_t[:, :, fc * FC:(fc + 1) * FC],
                    in_=dview(x_loc)[:, :, fc * FC:(fc + 1) * FC])

            stats1 = spool.tile([128, 8], f32, tag="stats1")
            se_parts = spool.tile([128, 16], f32, tag="separts")
            se_loc = spool.tile([128, CO], f32, tag="seloc")

            ps_cv = tc.alloc_tile_pool(name="pscv", bufs=2, space="PSUM")
            p_scr = tc.alloc_tile_pool(name="scrp", bufs=2)

            # ======== wy conv (bf16) + BN partials + AR1 ========
            for fc in range(4):
                for co in range(CO):
                    ps = ps_cv.tile([128, 512], f32, tag="c")
                    for ci in range(CI):
                        nc.tensor.matmul(
                            ps[:, :FC],
                            wt_z[:, ci, co * 128:(co + 1) * 128],
                            xloc_t[:, ci, fc * FC:(fc + 1) * FC],
                            start=(ci == 0), stop=(ci == CI - 1))
                    nc.vector.tensor_scalar_add(
                        wy_bf[:, co, fc * FC:(fc + 1) * FC], ps[:, :FC],
                        bz_t[:, co:co + 1])
            for co in range(CO):
                nc.vector.reduce_sum(stats1[:, co:co + 1], wy_bf[:, co, :],
                                     axis=AX)
                scr = p_scr.tile([128, T_LOC], bf16, tag="scr")
                nc.scalar.activation(scr[:], wy_bf[:, co, :], SQUARE,
                                     accum_out=stats1[:, 4 + co:5 + co])
            # Everything touching the collectives lives on the gpsimd queue as
            # one dependency chain, so a semaphore wait can never block an
            # unrelated engine queue (the scheduler may hoist within queues).
            nc.gpsimd.dma_start(out=cc1_in[:], in_=stats1[:])
            nc.gpsimd.collective_compute(
                "AllReduce", ADD,
                replica_groups=[[0, 1, 2, 3, 4, 5, 6, 7]],
                ins=[cc1_in[:]], outs=[cc1_out[:]])
            # BN finalize (gpsimd ALU): mu/var from AR1, rstd via Newton from
            # a host seed (no scalar-engine Sqrt -> no scalar-queue stall),
            # then wyfin = wy*alpha + beta in place.
            cnt = 1.0 / (N_B * THW)
            gst1 = spool.tile([128, 8], f32, tag="gst1")
            nc.gpsimd.dma_start(out=gst1[:], in_=cc1_out[:])
            mu = spool.tile([128, CO], f32, tag="mu")
            nc.gpsimd.tensor_scalar_mul(mu[:], gst1[:, 0:CO], cnt)
            nc.gpsimd.tensor_sub(mu[:], mu[:], bzc_t[:, 0:CO])
            ex2 = spool.tile([128, CO], f32, tag="ex2")
            nc.gpsimd.tensor_scalar_mul(ex2[:], gst1[:, CO:2 * CO], cnt)
            nc.gpsimd.tensor_sub(ex2[:], ex2[:], bzc_t[:, CO:2 * CO])
            var = spool.tile([128, CO], f32, tag="var")
            nc.gpsimd.tensor_mul(var[:], mu[:], mu[:])
            nc.gpsimd.tensor_sub(var[:], ex2[:], var[:])
            nc.gpsimd.tensor_scalar_add(var[:], var[:], BN_EPS)
            y_t = spool.tile([128, CO], f32, tag="rstd")
            nc.gpsimd.tensor_copy(y_t[:], rstd0_t[:])
            tnw = spool.tile([128, CO], f32, tag="tnw")
            for _ in range(4):
                nc.gpsimd.tensor_mul(tnw[:], y_t[:], y_t[:])
                nc.gpsimd.tensor_mul(tnw[:], tnw[:], var[:])
                nc.gpsimd.tensor_scalar(tnw[:], tnw[:], -0.5, 1.5,
                                        op0=MUL, op1=ADD)
                nc.gpsimd.tensor_mul(y_t[:], y_t[:], tnw[:])
            alpha = spool.tile([128, CO], f32, tag="alpha")
            nc.gpsimd.tensor_mul(alpha[:], y_t[:], bnw_t[:])
            beta = spool.tile([128, CO], f32, tag="beta")
            nc.gpsimd.tensor_mul(beta[:], mu[:], alpha[:])
            nc.gpsimd.tensor_sub(beta[:], bnb_t[:], beta[:])
            for co in range(CO):
                nc.gpsimd.tensor_scalar(
                    wy_bf[:, co, :], wy_bf[:, co, :],
                    alpha[:, co:co + 1], beta[:, co:co + 1],
                    op0=MUL, op1=ADD)

            # ======== K/V conv piece loop (x on sync, mask on vector) ========
            p_piece = tc.alloc_tile_pool(name="piecep", bufs=2)
            o = 0
            pieces = []
            while o < NST:
                w = min(4, NST - o)
                pieces.append((o, w))
                o += w
            for (pt0, ptw) in pieces:
                s_off = pt0 * 128
                pw = ptw * 128
                xp = p_piece.tile([128, CI, 512], bf16, tag="xp", name="xp")
                nc.sync.dma_start(
                    out=xp[:, :, :pw],
                    in_=dview(x_full)[:, :, s_off:s_off + pw])
                for co in range(CO):
                    ps = ps_cv.tile([128, 512], f32, tag="c")
                    for ci in range(CI):
                        nc.tensor.matmul(
                            ps[:, :pw],
                            wt_g[:, ci, co * 128:(co + 1) * 128],
                            xp[:, ci, :pw],
                            start=(ci == 0), stop=(ci == CI - 1))
                    nc.vector.tensor_scalar_add(
                        pgh[:, co, s_off:s_off + pw], ps[:, :pw],
                        bg_t[:, co:co + 1])
                mp = p_piece.tile([128, CI, 512], bf16, tag="mp", name="mp")
                nc.scalar.dma_start(
                    out=mp[:, :, :pw],
                    in_=dview(mask_full)[:, :, s_off:s_off + pw])
                for sj in range(ptw):
                    st = pt0 + sj
                    ps = ps_cv.tile([128, 512], f32, tag="c")
                    for ci in range(CI):
                        nc.tensor.matmul(
                            ps[:],
                            mp[:, ci, sj * 128:(sj + 1) * 128],
                            wt_h[:, ci, :],
                            start=(ci == 0), stop=(ci == CI - 1))
                    nc.vector.tensor_add(phmh[:, st, :], ps[:], bh_row[:])

            # ======== Q conv (fp8/bf16 out) + pm conv (bf16 out) ========
            for co in range(CO):
                for fc in range(4):
                    ps = ps_cv.tile([128, 512], f32, tag="c")
                    for ci in range(CI):
                        nc.tensor.matmul(
                            ps[:, :FC],
                            wt_h[:, ci, co * 128:(co + 1) * 128],
                            xloc_t[:, ci, fc * FC:(fc + 1) * FC],
                            start=(ci == 0), stop=(ci == CI - 1))
                    nc.vector.tensor_scalar_add(
                        phx[:, co, fc * FC:(fc + 1) * FC], ps[:, :FC],
                        bh_t[:, co:co + 1])
            for co in range(CO):
                for fc in range(4):
                    ps = ps_cv.tile([128, 512], f32, tag="c")
                    for ci in range(CI):
                        nc.tensor.matmul(
                            ps[:, :FC],
                            wt_m[:, ci, co * 128:(co + 1) * 128],
                            xloc_t[:, ci, fc * FC:(fc + 1) * FC],
                            start=(ci == 0), stop=(ci == CI - 1))
                    nc.vector.tensor_scalar_add(
                        pm_bf[:, co, fc * FC:(fc + 1) * FC], ps[:, :FC],
                        bm_t[:, co:co + 1])

            p_piece.release()
            p_scr.release()
            ps_cv.release()
            p_xl.release()
            p_w.release()

            # ======== attention: 4 t-blocks x 49 s-tiles, one pass ========
            p_expz = tc.alloc_tile_pool(name="expzp", bufs=1)
            expz = p_expz.tile([128, CO, T_LOC], bf16, tag="expz")
            ps_att = tc.alloc_tile_pool(name="psatt", bufs=1, space="PSUM")
            p_pt = tc.alloc_tile_pool(name="ptp", bufs=3)
            p_z = tc.alloc_tile_pool(name="zp", bufs=2)
            p_rb = tc.alloc_tile_pool(name="rbp", bufs=2)
            p_rr = tc.alloc_tile_pool(name="rrp", bufs=2)

            for bi, (t0, tfree) in enumerate(TBLOCKS):
                ocs = [ps_att.tile([128, 512], f32, tag=f"o{j}",
                                   name=f"o{j}_{bi}") for j in range(CO)]
                rps = ps_att.tile([1, 512], f32, tag="r", name=f"r{bi}")

                def emit_qk(st):
                    eps_t = ps_att.tile([128, 512], f32, tag="e", bufs=2,
                                        name=f"e{bi}_{st}")
                    if use_fp8:
                        for p2 in range(0, CI, 2):
                            nc.tensor.matmul(
                                eps_t[:, :tfree],
                                pgh[:, p2:p2 + 2, st * 128:(st + 1) * 128],
                                phx[:, p2:p2 + 2, t0:t0 + tfree],
                                start=(p2 == 0), stop=(p2 == CI - 2),
                                perf_mode=DR)
                    else:
                        for ci in range(CI):
                            nc.tensor.matmul(
                                eps_t[:, :tfree],
                                pgh[:, ci, st * 128:(st + 1) * 128],
                                phx[:, ci, t0:t0 + tfree],
                                start=(ci == 0), stop=(ci == CI - 1))
                    ptile = p_pt.tile([128, 512], bf16, tag="pt",
                                      name=f"pt{bi}_{st}")
                    nc.scalar.activation(ptile[:, :tfree], eps_t[:, :tfree],
                                         EXP, bias=m1b[:], scale=1.0)
                    return ptile

                nxt = emit_qk(0)
                for st in range(NST):
                    ptile = nxt
                    if st + 1 < NST:
                        nxt = emit_qk(st + 1)
                    for co in range(CO):
                        nc.tensor.matmul(
                            ocs[co][:, :tfree],
                            phmh[:, st, co * 128:(co + 1) * 128],
                            ptile[:, :tfree],
                            start=(st == 0), stop=(st == NST - 1))
                    nc.tensor.matmul(
                        rps[:, :tfree],
                        ones_t[:],
                        ptile[:, :tfree],
                        start=(st == 0), stop=(st == NST - 1))

                # block epilogue: rb = 1/(r + hmask) broadcast, z, exp, se
                rrow = p_rr.tile([1, 512], f32, tag="rrow", name=f"rrow{bi}")
                nc.vector.tensor_add(rrow[0:1, :tfree], rps[0:1, :tfree],
                                     hmask_t[0:1, t0:t0 + tfree])
                rrbf = p_rr.tile([1, 512], bf16, tag="rrbf", name=f"rrbf{bi}")
                with nc.allow_low_precision(reason="1/r broadcast in bf16"):
                    nc.vector.reciprocal(rrbf[0:1, :tfree], rrow[0:1, :tfree])
                rbb = ps_att.tile([128, 512], f32, tag="rbb", name=f"rbb{bi}")
                nc.tensor.matmul(rbb[:, :tfree], onesrow_t[0:1, :],
                                 rrbf[0:1, :tfree], start=True, stop=True)
                rb_sb = p_rb.tile([128, 512], bf16, tag="rb", name=f"rb{bi}")
                nc.vector.tensor_copy(rb_sb[:, :tfree], rbb[:, :tfree])
                for co in range(CO):
                    z_sb = p_z.tile([128, 512], f32, tag="z",
                                    name=f"z{bi}_{co}")
                    nc.vector.tensor_mul(z_sb[:, :tfree], ocs[co][:, :tfree],
                                         rb_sb[:, :tfree])
                    nc.scalar.activation(
                        expz[:, co, t0:t0 + tfree], z_sb[:, :tfree],
                        EXP, bias=m2b[:], scale=1.0,
                        accum_out=se_parts[:, co * 4 + bi:co * 4 + bi + 1])

            p_rr.release()
            p_rb.release()
            p_z.release()
            p_pt.release()
            ps_att.release()

            # ======== tail: AR2 + fuse ========
            for co in range(CO):
                nc.vector.reduce_sum(se_loc[:, co:co + 1],
                                     se_parts[:, co * 4:(co + 1) * 4], axis=AX)
            nc.vector.tensor_sub(se_loc[:], se_loc[:], secorr_t[:])
            nc.gpsimd.dma_start(out=cc2_in[:], in_=se_loc[:])
            nc.gpsimd.collective_compute(
                "AllReduce", ADD,
                replica_groups=[[0, 1, 2, 3], [4, 5, 6, 7]],
                ins=[cc2_in[:]], outs=[cc2_out[:]])

            # mt0 = expz * pm, independent of AR2 (fills the wait)
            for co in range(CO):
                nc.vector.tensor_mul(expz[:, co, :], expz[:, co, :],
                                     pm_bf[:, co, :])

            gst2 = spool.tile([128, CO], f32, tag="gst2")
            nc.gpsimd.dma_start(out=gst2[:], in_=cc2_out[:])
            gse = spool.tile([128, CO], f32, tag="gse")
            nc.vector.reciprocal(gse[:], gst2[:])
            nc.vector.tensor_scalar_mul(gse[:], gse[:], gamma)

            p_out = tc.alloc_tile_pool(name="outp", bufs=2)
            for co in range(CO):
                ot = p_out.tile([128, T_LOC], bf16, tag="ot")
                nc.vector.tensor_scalar_mul(ot[:], expz[:, co, :],
                                            gse[:, co:co + 1])
                nc.vector.tensor_add(ot[:], ot[:], wy_bf[:, co, :])
                nc.sync.dma_start(out=dview(out_loc)[:, co, :], in_=ot[:])
            p_out.release()
            p_expz.release()
            p_wyp.release()
            p_kv.release()
            p_phx.release()

    nc.compile()
    return nc


def _prepare_maps(x, mask, Wh, bh, Wg, bg, Wm, bm, Wz, bz, bn_w, bn_b, m2r):
    import ml_dtypes
    bf16 = ml_dtypes.bfloat16

    xf = np.ascontiguousarray(x.reshape(N_B, C, THW), dtype=np.float32)
    mf = np.ascontiguousarray(mask.reshape(N_B, C, THW), dtype=np.float32)

    def chunked_bias(b):
        return np.ascontiguousarray(b.reshape(CO, 128).T, dtype=np.float32)

    wht = np.ascontiguousarray(Wh.T).astype(bf16)
    wgt = np.ascontiguousarray(Wg.T).astype(bf16)
    wmt = np.ascontiguousarray(Wm.T).astype(bf16)
    wzt = np.ascontiguousarray(Wz.T).astype(bf16)
    bh_row = np.broadcast_to(bh.astype(np.float32), (128, C)).copy()
    ones_bf = np.ones((128, 1), dtype=bf16)
    onesrow_bf = np.ones((1, 128), dtype=bf16)

    # BN bias compensation: raw sums include (8*T_LOC - N*THW) padded columns
    # where wy == bz exactly (x padded with zeros).
    n_pad = 8 * T_LOC - N_B * THW
    cntf = 1.0 / (N_B * THW)
    bzc = np.zeros((128, 8), np.float32)
    bzc[:, 0:4] = chunked_bias(bz * (n_pad * cntf))
    bzc[:, 4:8] = chunked_bias((bz * bz) * (n_pad * cntf))

    # Newton seed for 1/sqrt(BN var): sampled estimate, refined on device.
    xs = np.concatenate([xf[n][:, ::11] for n in range(N_B)], axis=1)
    wys = (Wz.astype(np.float32) @ xs) + bz[:, None]
    var_est = wys.var(axis=1) + BN_EPS
    rstd0 = chunked_bias(1.0 / np.sqrt(var_est))

    in_maps = []
    for core in range(8):
        n, q = divmod(core, 4)
        t0 = T_LOC * q
        valid = int(np.clip(THW - t0, 0, T_LOC))
        x_locc = np.zeros((C, T_LOC), bf16)
        x_locc[:, :valid] = xf[n][:, t0:t0 + valid].astype(bf16)
        # hmask: tiny eps on valid t, huge on padded t so rb = 1/(r+hmask) ~ 0
        hmask = np.full((1, T_LOC), 1e30, np.float32)
        hmask[0, :valid] = R_EPS
        # padded t columns contribute exp(0 - m2) each to the se sums
        secorr = np.full((128, CO), (T_LOC - valid) * np.exp(-m2r), np.float32)
        in_maps.append(dict(
            x_full=xf[n].astype(bf16), mask_full=mf[n].astype(bf16),
            x_loc=x_locc,
            wht=wht, wgt=wgt, wmt=wmt, wzt=wzt,
            bh_in=chunked_bias(bh), bg_in=chunked_bias(bg),
            bm_in=chunked_bias(bm), bz_in=chunked_bias(bz),
            bh_row_in=bh_row,
            bnw_in=chunked_bias(bn_w), bnb_in=chunked_bias(bn_b),
            ones_in=ones_bf, onesrow_in=onesrow_bf,
            hmask_in=hmask, secorr_in=secorr, bzc_in=bzc,
            rstd0_in=rstd0,
        ))
    return in_maps


def _estimate_shifts(xf, mf, Wh, bh, Wg, bg):
    # M1: safe global upper-bound estimate for the max of the energy matrix.
    # Any constant shift cancels exactly in softmax; the +5 margin absorbs
    # sampling misses and fp8 quantization noise.
    ti = np.arange(0, THW, 41)
    si = np.arange(0, THW, 7)
    m_s = -np.inf
    for n in range(N_B):
        Q = (Wh @ xf[n][:, ti]) + bh[:, None]
        K = (Wg @ xf[n][:, si]) + bg[:, None]
        m_s = max(m_s, float((Q.T @ K).max()))
    m1 = m_s + 5.0
    # M2: norm bound on |ph_m| entries (second softmax argument is a convex
    # combination of ph_m values, so bounded by max |ph_m|).
    whn = float(np.linalg.norm(Wh, axis=1).max())
    mcn = max(float(np.linalg.norm(mf[n], axis=0).max()) for n in range(N_B))
    m2 = whn * mcn + float(np.abs(bh).max()) + 1.0
    return m1, m2


def kernel(x, mask, Wh, bh, Wg, bg, Wm, bm, Wz, bz, bn_w, bn_b, gamma,
           _debug=False, _trace=False):
    from concourse.bass_utils import run_bass_kernel_spmd

    x = np.asarray(x, np.float32)
    mask = np.asarray(mask, np.float32)
    Wh = np.asarray(Wh, np.float32); bh = np.asarray(bh, np.float32)
    Wg = np.asarray(Wg, np.float32); bg = np.asarray(bg, np.float32)
    Wm = np.asarray(Wm, np.float32); bm = np.asarray(bm, np.float32)
    Wz = np.asarray(Wz, np.float32); bz = np.asarray(bz, np.float32)
    bn_w = np.asarray(bn_w, np.float32); bn_b = np.asarray(bn_b, np.float32)
    gammaf = float(np.asarray(gamma))
    use_fp8 = os.environ.get("BASS_NO_FP8", "0") != "1"

    xf = x.reshape(N_B, C, THW)
    mf = mask.reshape(N_B, C, THW)
    m1, m2 = _estimate_shifts(xf, mf, Wh, bh, Wg, bg)
    key = (round(m1, 1), round(m2, 1), round(gammaf, 6), use_fp8)
    if key not in _PROG_CACHE:
        _PROG_CACHE[key] = _build_program(key[0], key[1], gammaf,
                                          use_fp8=use_fp8)
    nc = _PROG_CACHE[key]

    in_maps = _prepare_maps(x, mask, Wh, bh, Wg, bg, Wm, bm, Wz, bz,
                            bn_w, bn_b, key[1])
    res = run_bass_kernel_spmd(nc, in_maps, core_ids=list(range(8)),
                               trace=_trace)

    out = np.empty((N_B, C, THW), np.float32)
    for core in range(8):
        n, q = divmod(core, 4)
        t0 = T_LOC * q
        valid = int(np.clip(THW - t0, 0, T_LOC))
        if valid > 0:
            out[n][:, t0:t0 + valid] = (
                res.results[core]["out_loc"][:, :valid].astype(np.float32))
    out = out.reshape(N_B, C, T, H, W)
    if _debug or _trace:
        return out, res
    return out
